# revision 10
# baseline (speedup 1.0000x reference)
"""Multi-head self-attention Trainium2 kernel (8 NeuronCores, head-parallel).

Problem: L=4096, F_IN=1024, H=16, DH=64, F_OUT=1024, fp32.
Sharding: 2 heads per core (tensor parallel over heads). Each core computes
its 2 heads' attention and partial output projection; host sums partials.

Per-core pipeline (all matmuls fp32r = TF32-like, fp32 accumulate):
  1. qT,kT [128,4096] = W.T @ x.T (needs xT; host pre-transposes x).
     v natural [4096,128] via x-tiles as stationary operand.
  2. Per i-chunk (512 cols), per j-tile (128): scoresT both heads
     (row-packed K=64 matmuls) -> exp (ACT, scale=1/8) -> attn@v with a
     ones-column on the stationary operand so PSUM row 64 accumulates the
     softmax denominators.
  3. reciprocal + partition-broadcast of denominators, normalize vals,
     output projection (both heads accumulate into one PSUM), DMA out.

Bias handling: bq/bk folded into ACT bias at qT/kT evacuation; bv is exact
as a host-side constant (softmax rows sum to 1 => out += sum_h bv_h @ Wo_h);
bo added on host.
"""

import numpy as np

L, F_IN, H, DH, F_OUT = 4096, 1024, 16, 64, 1024
NCORES = 8
HPC = H // NCORES  # heads per core = 2
D2 = HPC * DH      # 128, per-core packed head dim

_BUILT = None


def _attention(nc, tc, ps2s, ps2v, pe, qT, kT, vx0, vx1, vals0, vals1,
               NI, NJ, F, FR, Act):
    DHL = DH
    for ic in range(NI):
        i0 = ic * 512
        pv0 = ps2v.tile([DHL + 1, 512], F, tag="pv0")
        pv1 = ps2v.tile([DHL + 1, 512], F, tag="pv1")
        for jt in range(NJ):
            j0 = jt * 128
            ps = ps2s.tile([128, 1024], F, tag="pss")
            nc.tensor.matmul(
                ps[:, 0:512], kT[0:64, j0:j0 + 128],
                qT[0:64, i0:i0 + 512],
                start=True, stop=True, tile_position=(0, 0),
            )
            nc.tensor.matmul(
                ps[:, 512:1024], kT[64:128, j0:j0 + 128],
                qT[64:128, i0:i0 + 512],
                start=True, stop=True, tile_position=(64, 0),
            )
            eT = pe.tile([128, 1024], FR, tag="eT")
            nc.scalar.activation(eT[:], ps[:], Act.Exp, scale=0.125)
            nc.tensor.matmul(
                pv0[:], vx0[:, jt, :], eT[:, 0:512],
                start=(jt == 0), stop=(jt == NJ - 1),
            )
            nc.tensor.matmul(
                pv1[:], vx1[:, jt, :], eT[:, 512:1024],
                start=(jt == 0), stop=(jt == NJ - 1),
            )
        nc.vector.tensor_copy(vals0[:, ic, :], pv0[:])
        nc.vector.tensor_copy(vals1[:, ic, :], pv1[:])


def _build():
    import concourse.bass as bass
    import concourse.mybir as mybir
    import concourse.tile as tile
    from concourse import bacc

    F = mybir.dt.float32
    FR = mybir.dt.float32r
    Act = mybir.ActivationFunctionType

    nc = bacc.Bacc("TRN2", target_bir_lowering=False, debug=False)

    xT_d = nc.declare_dram_parameter("xT", [F_IN, L], F, isOutput=False)
    wq_d = nc.declare_dram_parameter("wq", [F_IN, D2], F, isOutput=False)
    wk_d = nc.declare_dram_parameter("wk", [F_IN, D2], F, isOutput=False)
    wv_d = nc.declare_dram_parameter("wv", [F_IN, D2], F, isOutput=False)
    bq_d = nc.declare_dram_parameter("bq", [D2], F, isOutput=False)
    bk_d = nc.declare_dram_parameter("bk", [D2], F, isOutput=False)
    wo0_d = nc.declare_dram_parameter("wo0", [DH, F_OUT], F, isOutput=False)
    wo1_d = nc.declare_dram_parameter("wo1", [DH, F_OUT], F, isOutput=False)
    out_d = nc.declare_dram_parameter("out", [L, F_OUT], F, isOutput=True)
    import os
    dbg = bool(os.environ.get("K_DEBUG"))
    if dbg:
        dbg_q = nc.declare_dram_parameter("dbg_q", [128, L], F, isOutput=True)
        dbg_k = nc.declare_dram_parameter("dbg_k", [128, L], F, isOutput=True)
        dbg_v = nc.declare_dram_parameter("dbg_v", [128, 32 * 65], F, isOutput=True)
        dbg_s = nc.declare_dram_parameter("dbg_s", [65, 8 * 512], F, isOutput=True)
        dbg_rb = nc.declare_dram_parameter("dbg_rb", [64, 8 * 512], F, isOutput=True)

    KT = F_IN // 128   # 8 f-tiles
    NI = L // 512      # 8 i-chunks
    NJ = L // 128      # 32 j-tiles
    HL = L // 2        # 2048, half of L for the two-pass phase 1

    with tile.TileContext(nc) as tc:
        with tc.tile_pool(name="persist", bufs=1) as pp:
            # persistent SBUF tensors
            qT = pp.tile([128, L], FR, tag="qT")       # [d2, i]
            kT = pp.tile([128, L], FR, tag="kT")       # [d2, j]
            vx0 = pp.tile([128, NJ, DH + 1], FR, tag="vx0")  # [j_in, jt, d|ones]
            vx1 = pp.tile([128, NJ, DH + 1], FR, tag="vx1")
            wo0 = pp.tile([DH, F_OUT], FR, tag="wo0")
            wo1 = pp.tile([DH, F_OUT], FR, tag="wo1")
            bq = pp.tile([128, 1], F, tag="bq")
            bk = pp.tile([128, 1], F, tag="bk")
            ones32 = pp.tile([128, NJ], F, tag="ones32")

            nc.sync.dma_start(out=wo0[:], in_=wo0_d.ap().bitcast(FR))
            nc.sync.dma_start(out=wo1[:], in_=wo1_d.ap().bitcast(FR))
            nc.sync.dma_start(out=bq[:], in_=bq_d.ap()[:, None])
            nc.sync.dma_start(out=bk[:], in_=bk_d.ap()[:, None])
            nc.vector.memset(ones32[:], 1.0)
            # ones column of the v-extended stationary operand
            nc.vector.tensor_copy(vx0[:, :, DH:DH + 1], ones32[:, :, None])
            nc.vector.tensor_copy(vx1[:, :, DH:DH + 1], ones32[:, :, None])

            # ---- Phase 1: QKV projections (two halves of L) ----
            with tc.tile_pool(name="ph1", bufs=1) as p1, \
                 tc.tile_pool(name="ph1w", bufs=1) as p1w, \
                 tc.tile_pool(name="ps1", bufs=2, space="PSUM") as ps1:
                wq = p1w.tile([128, KT, D2], FR, tag="wq")
                wk = p1w.tile([128, KT, D2], FR, tag="wk")
                wv = p1w.tile([128, KT, D2], FR, tag="wv")
                for wt, wd in ((wq, wq_d), (wk, wk_d), (wv, wv_d)):
                    nc.sync.dma_start(
                        out=wt[:],
                        in_=wd.ap().rearrange("(k p) d -> p k d", p=128).bitcast(FR),
                    )

                for half in range(2):
                    l0 = half * HL
                    xt = p1.tile([128, KT, HL], FR, tag="xt")
                    for kt in range(KT):
                        nc.sync.dma_start(
                            out=xt[:, kt, :],
                            in_=xT_d.ap()[kt * 128:(kt + 1) * 128,
                                          l0:l0 + HL].bitcast(FR),
                        )
                    # qT / kT chunks of this half
                    for ch in range(HL // 512):
                        c0 = ch * 512
                        g0 = l0 + c0
                        for wt, dst, bias in ((wq, qT, bq), (wk, kT, bk)):
                            ps = ps1.tile([128, 512], F, tag="psqk")
                            for kt in range(KT):
                                nc.tensor.matmul(
                                    ps[:], wt[:, kt, :], xt[:, kt, c0:c0 + 512],
                                    start=(kt == 0), stop=(kt == KT - 1),
                                )
                            nc.scalar.activation(
                                dst[:, g0:g0 + 512], ps[:], Act.Identity,
                                bias=bias[:], scale=1.0,
                            )
                    # v natural for this half's j-tiles
                    for jl in range(HL // 128):
                        jt = half * (HL // 128) + jl
                        ps = ps1.tile([128, D2], F, tag="psv")
                        for kt in range(KT):
                            nc.tensor.matmul(
                                ps[:], xt[:, kt, jl * 128:(jl + 1) * 128],
                                wv[:, kt, :],
                                start=(kt == 0), stop=(kt == KT - 1),
                            )
                        nc.vector.tensor_copy(vx0[:, jt, 0:DH], ps[:, 0:DH])
                        nc.vector.tensor_copy(vx1[:, jt, 0:DH], ps[:, DH:D2])

            # ---- Phase 2: attention ----
            with tc.tile_pool(name="ph2", bufs=1) as p2:
                vals0 = p2.tile([DH + 1, NI, 512], FR, tag="vals0")
                vals1 = p2.tile([DH + 1, NI, 512], FR, tag="vals1")

                with tc.tile_pool(name="expp", bufs=3) as pe, \
                     tc.tile_pool(name="ps2s", bufs=2, space="PSUM") as ps2s, \
                     tc.tile_pool(name="ps2v", bufs=2, space="PSUM") as ps2v:
                    _attention(nc, tc, ps2s, ps2v, pe,
                               qT, kT, vx0, vx1, vals0, vals1,
                               NI, NJ, F, FR, Act)
                if dbg:
                    nc.sync.dma_start(out=dbg_q.ap(), in_=qT[:].bitcast(F))
                    nc.sync.dma_start(out=dbg_k.ap(), in_=kT[:].bitcast(F))
                    nc.sync.dma_start(
                        out=dbg_v.ap(),
                        in_=vx0[:].bitcast(F).rearrange("p a b -> p (a b)"))
                    nc.sync.dma_start(
                        out=dbg_s.ap(),
                        in_=vals0[:].bitcast(F).rearrange("p a b -> p (a b)"))

                # ---- Phase 3: normalize + output projection ----
                with tc.tile_pool(name="ph3", bufs=1) as p3, \
                     tc.tile_pool(name="outp", bufs=4) as po, \
                     tc.tile_pool(name="ps3", bufs=2, space="PSUM") as ps3:
                    rb0 = p3.tile([DH, NI, 512], F, tag="rb0")
                    rb1 = p3.tile([DH, NI, 512], F, tag="rb1")
                    rc0 = p3.tile([1, NI, 512], F, tag="rc0")
                    rc1 = p3.tile([1, NI, 512], F, tag="rc1")
                    sh0 = p3.tile([1, NI, 512], F, tag="sh0")
                    sh1 = p3.tile([1, NI, 512], F, tag="sh1")
                    for vals, rb, rc, sh in ((vals0, rb0, rc0, sh0),
                                             (vals1, rb1, rc1, sh1)):
                        # sums live on partition 64; shift to partition 0
                        nc.sync.dma_start(
                            out=sh[:], in_=vals[DH:DH + 1, :, :].bitcast(F))
                        nc.vector.reciprocal_approx_fast(out=rc[:], in_=sh[:])
                        nc.gpsimd.partition_broadcast(rb[:], rc[:], channels=DH)
                        nc.vector.tensor_mul(
                            vals[0:DH, :, :], vals[0:DH, :, :], rb[:],
                        )
                    if dbg:
                        nc.sync.dma_start(
                            out=dbg_rb.ap(),
                            in_=rb0[:].rearrange("p a b -> p (a b)"))

                    for it in range(L // 128):
                        ic, iw = divmod(it, 4)
                        isl = slice(iw * 128, (iw + 1) * 128)
                        for fc in range(F_OUT // 512):
                            f0 = fc * 512
                            ps = ps3.tile([128, 512], F, tag="pso")
                            nc.tensor.matmul(
                                ps[:], vals0[0:DH, ic, isl],
                                wo0[:, f0:f0 + 512], start=True, stop=False,
                            )
                            nc.tensor.matmul(
                                ps[:], vals1[0:DH, ic, isl],
                                wo1[:, f0:f0 + 512], start=False, stop=True,
                            )
                            ot = po.tile([128, 512], F, tag="ot")
                            nc.any.tensor_copy(ot[:], ps[:])
                            nc.sync.dma_start(
                                out=out_d.ap()[it * 128:(it + 1) * 128,
                                               f0:f0 + 512],
                                in_=ot[:],
                            )

    nc.compile()
    return nc


def _get_built():
    global _BUILT
    if _BUILT is None:
        _BUILT = _build()
    return _BUILT


def kernel(x, Wq, bq, Wk, bk, Wv, bv, Wo, bo):
    from concourse.bass_utils import run_bass_kernel_spmd

    x = np.ascontiguousarray(np.asarray(x, dtype=np.float32))
    Wq = np.asarray(Wq, dtype=np.float32)
    Wk = np.asarray(Wk, dtype=np.float32)
    Wv = np.asarray(Wv, dtype=np.float32)
    Wo = np.asarray(Wo, dtype=np.float32)
    bq = np.asarray(bq, dtype=np.float32)
    bk = np.asarray(bk, dtype=np.float32)
    bv = np.asarray(bv, dtype=np.float32)
    bo = np.asarray(bo, dtype=np.float32)

    nc = _get_built()

    xT = np.ascontiguousarray(x.T)  # [F_IN, L]
    in_maps = []
    for c in range(NCORES):
        hs = slice(c * HPC, (c + 1) * HPC)
        in_maps.append({
            "xT": xT,
            "wq": np.ascontiguousarray(Wq[:, hs, :].reshape(F_IN, D2)),
            "wk": np.ascontiguousarray(Wk[:, hs, :].reshape(F_IN, D2)),
            "wv": np.ascontiguousarray(Wv[:, hs, :].reshape(F_IN, D2)),
            "bq": np.ascontiguousarray(bq[hs].reshape(D2)),
            "bk": np.ascontiguousarray(bk[hs].reshape(D2)),
            "wo0": np.ascontiguousarray(Wo[c * HPC]),
            "wo1": np.ascontiguousarray(Wo[c * HPC + 1]),
        })

    res = run_bass_kernel_spmd(nc, in_maps, list(range(NCORES)))
    acc = np.zeros((L, F_OUT), dtype=np.float64)
    for c in range(NCORES):
        acc += res.results[c]["out"].astype(np.float64)
    # bv contribution (softmax rows sum to 1) + bo, both exact on host
    acc += (bv.reshape(1, H * DH).astype(np.float64)
            @ Wo.reshape(H * DH, F_OUT).astype(np.float64))
    acc += bo.astype(np.float64)
    return acc.astype(np.float32)
